# revision 39
# baseline (speedup 1.0000x reference)
"""Trainium2 Bass kernel for nn_GAT_7507602833557 (8-core SPMD GAT).

Sharding: query-node rows split across 8 cores (512 rows each); keys/values
replicated. Per-core adjacency slice is passed pre-transposed ([keys, own
queries]) in bf16 ({0,1} values are exact in bf16).

Math notes (per attention map, 9 maps total: 6 spatial + 2 intent + 1 output):
  e[i,j] = leakyrelu(f1[i] + f2[j], 0.2);  softmax over masked j; att @ V.
  exp(leakyrelu(z)) = max(exp(z), exp(0.2 z)) since z >= 0.2z for z>=0 and
  z <= 0.2z for z<0, and exp is monotone. Factorizing,
    exp(lrelu(f1+f2)) = e^{f1[i]} * e^{0.2 f2[j]} * max(e^{0.8 f2[j]}, e^{-0.8 f1[i]})
  and the e^{f1[i]} factor is constant along j so it cancels in the softmax
  normalization. With P=e^{0.8 f2}, C=e^{0.2 f2}, Q=e^{-0.8 f1} the masked
  unnormalized weight is  m[j,i] = adj[j,i] * max(P[j], Q[i]) * C[j],
  one dual-op tensor_scalar (max then mult, both per-partition scalars) plus
  one tensor_tensor mask multiply per tile. Softmax denominator via a ones
  column appended to the value matrix. elu(v) = min(exp(v)-1, max(v, 0)).

Schedule: heads run outermost; within a head the keys go in strips of STRIP
key-tiles (ts per tile at DVE 2x, mask-mult batched per strip at 2x). The
per-head accumulation is TRANSPOSED: one matmul per (head, key tile) with
the [128,33] value slice stationary and the [128,512] masked map moving
(222ns/mm streaming 512 cols vs 4 mms of 33 cols + full LDWEIGHTS each),
into a [33,512] PSUM accT; after the head, an ACT copy + 4 PE transposes
put it back in query-partition layout so the batched elu/div epilogue is
unchanged. Values (whp) are produced one head-PAIR ahead of consumption --
a freshly-written stationary serializes the consumer's LDWEIGHTS behind
the producing copy and throttles the PE below the DVE map rate. Q-row
broadcasts go through K=1 ones-matmuls (PSUM) instead of DMA roundtrips;
a dummy 16-element AllGather at kernel start absorbs the ~11us collective
firmware spin-up so the real payload AllGather doesn't pay it; whop is
re-loaded post-gather in 8 per-source-core chunks spread over the sync and
scalar DGE queues. ACT activation-table reloads (1.28us per Exp<->Copy
switch) are kept off the startup chain by ordering exp groups together.
All map ops stay on the DVE (gpsimd elementwise stalls the DVE via the
shared SBUF ports; gpsimd cannot touch PSUM at all).
"""
import os
import numpy as np

import concourse.bass as bass
import concourse.bacc as bacc
import concourse.tile as tile
from concourse import mybir
from concourse.bass_utils import run_bass_kernel_spmd
from concourse.masks import make_identity

import ml_dtypes

N, NIN, NHID, NOUT = 4096, 64, 32, 64
NHEADS, D_INT = 8, 32
H_SP, H_INT = 6, 2
NCORES = 8
R = N // NCORES           # 512 own query rows per core
JT = N // 128             # 32 key tiles
IT = R // 128             # 4 own query tiles
STRIP = int(os.environ.get("KERNEL_STRIP", "8"))   # key tiles per strip
NSTRIP = JT // STRIP
F32 = mybir.dt.float32
BF16 = mybir.dt.bfloat16
MAPDT = F32 if os.environ.get("KERNEL_F32") else BF16
NPMAP = np.float32 if os.environ.get("KERNEL_F32") else ml_dtypes.bfloat16


def _build_program(reps=1):
    nc = bacc.Bacc("TRN2", target_bir_lowering=False, debug=False,
                   num_devices=NCORES)
    d_x = nc.dram_tensor("xT", [NIN, N], MAPDT, kind="ExternalInput")
    d_ie = nc.dram_tensor("ieT", [D_INT, N], MAPDT, kind="ExternalInput")
    d_adjT = nc.dram_tensor("adjT", [N, R], MAPDT, kind="ExternalInput")
    d_xo = nc.dram_tensor("xoT", [NIN, R], MAPDT, kind="ExternalInput")
    d_io = nc.dram_tensor("ioT", [D_INT, R], MAPDT, kind="ExternalInput")
    d_wsp = nc.dram_tensor("wsp", [H_SP, NIN, NHID], MAPDT, kind="ExternalInput")
    d_asp = nc.dram_tensor("asp", [H_SP, 2 * NHID], MAPDT, kind="ExternalInput")
    d_wint = nc.dram_tensor("wint", [H_INT, NIN, NHID], MAPDT, kind="ExternalInput")
    d_aint = nc.dram_tensor("aint", [H_INT, 2 * D_INT], MAPDT, kind="ExternalInput")
    d_wout = nc.dram_tensor("wout", [NHEADS * NHID, NOUT], F32, kind="ExternalInput")
    d_aout = nc.dram_tensor("aout", [2 * NOUT], F32, kind="ExternalInput")
    d_out = nc.dram_tensor("out", [R, NOUT], F32, kind="ExternalOutput")
    if os.environ.get("KERNEL_DEBUG"):
        nc.dbg_et = nc.dram_tensor("dbg_et", [128, JT, 16], F32, kind="ExternalOutput")
        nc.dbg_qb = nc.dram_tensor("dbg_qb", [128, NHEADS, R], F32, kind="ExternalOutput")
        nc.dbg_acc = nc.dram_tensor("dbg_acc", [128, IT, NHEADS * (NHID + 1)], F32, kind="ExternalOutput")
    else:
        nc.dbg_et = nc.dbg_qb = nc.dbg_acc = None

    with tile.TileContext(nc) as tc:
        for _ in range(reps):
            _kernel_body(tc, d_x, d_ie, d_adjT, d_xo, d_io, d_wsp, d_asp,
                         d_wint, d_aint, d_wout, d_aout, d_out)
    nc.compile()
    return nc


def _kernel_body(tc, d_x, d_ie, d_adjT, d_xo, d_io, d_wsp, d_asp, d_wint,
                 d_aint, d_wout, d_aout, d_out):
    nc = tc.nc
    from contextlib import ExitStack
    ctx = ExitStack()
    big = ctx.enter_context(tc.tile_pool(name="big", bufs=1))
    work = ctx.enter_context(tc.tile_pool(name="work", bufs=int(os.environ.get("KERNEL_WORK_BUFS", str(16 // STRIP)))))
    mpool = ctx.enter_context(tc.tile_pool(name="mpool", bufs=int(os.environ.get("KERNEL_M_BUFS", str(64 // STRIP)))))
    psum = ctx.enter_context(tc.tile_pool(name="psum", bufs=2, space="PSUM"))
    pet = ctx.enter_context(tc.tile_pool(name="pet", bufs=1, space="PSUM"))
    pacc = ctx.enter_context(tc.tile_pool(name="pacc", bufs=1, space="PSUM"))
    ext = ctx.enter_context(tc.tile_pool(name="ext", bufs=2))
    dram = ctx.enter_context(tc.tile_pool(name="dram", bufs=1, space="DRAM"))
    try:
        _body(tc, ctx, big, work, psum, pacc, dram, d_x, d_ie, d_adjT, d_xo,
              d_io, d_wsp, d_asp, d_wint, d_aint, d_wout, d_aout, d_out, ext,
              mpool, pet)
    finally:
        ctx.close()


def _body(tc, ctx, big, work, psum, pacc, dram, d_x, d_ie, d_adjT, d_xo, d_io,
          d_wsp, d_asp, d_wint, d_aint, d_wout, d_aout, d_out, ext, mpool, pet):
    nc = tc.nc
    Act = mybir.ActivationFunctionType
    Alu = mybir.AluOpType

    # ---------------- loads (critical-path-first order) --------------------
    # Queue assignment = serialization domains (each engine's DGE queue is
    # in-order): sync gets only the tiny loads that gate the intent-head Q
    # rows; ieT rides alone on the vector queue; weights/xoT on scalar;
    # the bulk adj/x transfers on gpsimd.
    # warm-up collective: absorbs the ~11us collective-firmware spin-up
    # (TPB_TRIGGER -> ALGO_MESH_BEGIN) so the real AllGather doesn't pay it
    ccw_in = dram.tile([1, 16], MAPDT, tag="ccw_in")
    ccw_out = dram.tile([NCORES, 16], MAPDT, tag="ccw_out")
    nc.gpsimd.collective_compute(
        "AllGather", mybir.AluOpType.bypass,
        replica_groups=[list(range(NCORES))],
        ins=[ccw_in.opt()], outs=[ccw_out.opt()])
    aintp = big.tile([D_INT, 2 * H_INT], MAPDT, tag="aintp")
    nc.sync.dma_start(out=aintp, in_=d_aint.ap().rearrange("h (c o) -> o (h c)", c=2))
    ioT = big.tile([D_INT, R], MAPDT, tag="ioT")
    nc.sync.dma_start(out=ioT, in_=d_io.ap())
    apair = big.tile([NHID, 2 * H_SP], MAPDT, tag="apair")
    nc.sync.dma_start(out=apair, in_=d_asp.ap().rearrange("h (c o) -> o (h c)", c=2))
    # ieT in per-strip tiles (8 key tiles each) so the first ET strip is
    # gated on a 128KB transfer, not the full 512KB; first chunk rides the
    # short sync queue, the rest go on scalar.
    ieT_q = [big.tile([D_INT, N // 4], MAPDT, tag=f"ieT{g}", name=f"ieT_{g}")
             for g in range(4)]
    nc.scalar.dma_start(out=ieT_q[0], in_=d_ie.ap()[:, 0:1024])

    def ieT(jt):   # [32, 128] column block of intent embeds for key tile jt
        return ieT_q[jt // 8][:, 128 * (jt % 8):128 * (jt % 8 + 1)]
    w_all3 = big.tile([NIN, NHEADS, NHID], MAPDT, tag="w_all3")
    nc.scalar.dma_start(out=w_all3[:, 0:H_SP, :],
                        in_=d_wsp.ap().rearrange("h f o -> f h o"))
    nc.scalar.dma_start(out=w_all3[:, H_SP:, :],
                        in_=d_wint.ap().rearrange("h f o -> f h o"))
    w_all = w_all3.rearrange("f h o -> f (h o)")
    for g in range(1, 4):
        nc.scalar.dma_start(out=ieT_q[g],
                            in_=d_ie.ap()[:, 1024 * g:1024 * (g + 1)])
    xoT = big.tile([NIN, R], MAPDT, tag="xoT")
    nc.scalar.dma_start(out=xoT, in_=d_xo.ap())
    adjT_sb = big.tile([128, JT, R], MAPDT, tag="adjT_sb")

    def load_adj(g):
        nc.gpsimd.dma_start(
            out=adjT_sb[:, 4 * g:4 * (g + 1), :],
            in_=d_adjT.ap()[4 * g * 128:4 * (g + 1) * 128, :]
                .rearrange("(t p) i -> p t i", p=128))
    xT = big.tile([NIN, N], MAPDT, tag="xT")
    load_adj(0)
    nc.gpsimd.dma_start(out=xT[:, 0:2048], in_=d_x.ap()[:, 0:2048])
    load_adj(1)
    nc.gpsimd.dma_start(out=xT[:, 2048:4096], in_=d_x.ap()[:, 2048:4096])
    for g in range(2, 8):
        load_adj(g)
    wout_f = big.tile([128, 2, NOUT], F32, tag="wout_f")
    nc.gpsimd.dma_start(out=wout_f, in_=d_wout.ap().rearrange("(c p) o -> p c o", p=128))
    aout_sb = big.tile([NOUT, 2], F32, tag="aout_sb")
    nc.gpsimd.dma_start(out=aout_sb, in_=d_aout.ap().rearrange("(c o) -> o c", c=2))

    # ---------------- intent-head fast path (gates first DVE work) --------
    # aint_arr [32, 4]: 0:2 = 0.8*a2 (P), 2:4 = 0.2*a2 (C); aq [32,2] = -0.8*a1
    aint_arr = big.tile([D_INT, 2 * H_INT], MAPDT, tag="aint_arr")
    aq = big.tile([D_INT, H_INT], MAPDT, tag="aq")
    ai_hc = aintp[:].rearrange("f (h c) -> f c h", c=2)
    nc.scalar.mul(out=aint_arr[:, 0:H_INT], in_=ai_hc[:, 1, :], mul=0.8)
    nc.scalar.mul(out=aint_arr[:, H_INT:], in_=ai_hc[:, 1, :], mul=0.2)
    nc.scalar.mul(out=aq, in_=ai_hc[:, 0, :], mul=-0.8)
    qrow_in = big.tile([1, H_INT, R], MAPDT, tag="qrow_in")
    for h2 in range(H_INT):
        pq1 = psum.tile([1, R], F32, tag="ps")
        nc.tensor.matmul(pq1, aq[:, h2:h2 + 1], ioT)
        nc.scalar.activation(out=qrow_in[:, h2, :], in_=pq1, func=Act.Exp)
    qb = big.tile([128, NHEADS, R], MAPDT, tag="qb")
    ones1 = big.tile([1, 128], MAPDT, tag="ones1")
    nc.vector.memset(ones1, 1.0)
    HEADS = [6, 7, 0, 1, 2, 3, 4, 5]   # intent heads first (shortest dep chain)
    # intent ET per strip: separate tiles keep each strip's ts dependency
    # pinned to exactly its own exp (tile-granular dep tracking otherwise
    # serializes the first ts behind ALL exps)
    et_int = [big.tile([128, STRIP, 2 * H_INT], F32, tag=f"eti{s}",
                       name=f"et_int_{s}") for s in range(NSTRIP)]
    et_sp = [big.tile([128, STRIP, 2 * H_SP], F32, tag=f"etsp{s}",
                      name=f"et_sp_{s}") for s in range(NSTRIP)]
    # one 16-col PSUM tile shared by both ET matmul groups (fits one bank):
    # cols 0:12 spatial (psp), cols 12:16 intent (pint)
    pet16 = pet.tile([128, JT, 16], F32, tag="pet16")
    pint = pet16[:, :, 12:16]

    def et_int_strip(s):
        for jt in range(STRIP * s, STRIP * (s + 1)):
            nc.tensor.matmul(pint[:, jt, :], ieT(jt), aint_arr)
        nc.scalar.activation(out=et_int[s], in_=pint[:, STRIP * s:STRIP * (s + 1), :],
                             func=Act.Exp)

    # strip-0 exp BEFORE the qb copies: the ACT queue then runs
    # Exp(qrow),Exp(et0),Copy(qb) -- one activation-table reload (1.28us)
    # instead of two on the first map op's gating chain. Later strips go
    # after the copies (their ieT chunks land later; an in-order ACT queue
    # stalled on them would delay the qb copies).
    et_int_strip(0)
    # Q broadcast via K=1 PE matmul (ones column) + PSUM->SBUF copy: no DMA
    # queue involvement, so the first map op is not stuck behind bulk loads.
    for h in (6, 7):
        pqb = psum.tile([128, R], F32, tag="ps")
        nc.tensor.matmul(pqb, ones1, qrow_in[:, h - H_SP, :])
        nc.scalar.copy(out=qb[:, h, :], in_=pqb)
    for s in range(1, NSTRIP):
        et_int_strip(s)

    # ---------------- wtilde: spatial a-vectors pre-projected through W ----
    ident = big.tile([128, 128], F32, tag="ident")
    make_identity(nc, ident)
    if MAPDT == F32:
        id_map = ident
    else:
        ident_b = big.tile([128, 128], BF16, tag="ident_b")
        make_identity(nc, ident_b)
        id_map = ident_b

    def tr(out, in_, idt):
        p = in_.partition_size()
        nc.tensor.transpose(out, in_, idt[0:p, 0:p])

    wt = big.tile([NHID, H_SP, NIN], MAPDT, tag="wt")
    for grp in range(2):
        ptw = psum.tile([NHID, 3 * NIN], MAPDT, tag="ps")
        for k in range(3):
            h = 3 * grp + k
            tr(ptw[:, NIN * k:NIN * (k + 1)],
               w_all[:, NHID * h:NHID * (h + 1)], id_map)
        nc.scalar.copy(out=wt[:, 3 * grp:3 * grp + 3, :], in_=ptw)
    pw = psum.tile([NIN, 2 * H_SP], F32, tag="ps")
    for h in range(H_SP):
        nc.tensor.matmul(pw[:, 2 * h:2 * h + 2], wt[:, h, :],
                         apair[:, 2 * h:2 * h + 2])
    # wtilde [64, 12]: 0:6 = 0.8*w2 (P), 6:12 = 0.2*w2 (C); wq [64, 6] = -0.8*w1
    wtilde = big.tile([NIN, 2 * H_SP], MAPDT, tag="wtilde")
    wq = big.tile([NIN, H_SP], MAPDT, tag="wq")
    pw_hc = pw.rearrange("f (h c) -> f c h", c=2)
    w1cols = pw_hc[:, 0, :]
    w2cols = pw_hc[:, 1, :]
    nc.scalar.mul(out=wtilde[:, 0:H_SP], in_=w2cols, mul=0.8)
    nc.scalar.mul(out=wtilde[:, H_SP:], in_=w2cols, mul=0.2)
    nc.scalar.mul(out=wq, in_=w1cols, mul=-0.8)

    # ---------------- spatial Q rows + broadcast ---------------------------
    qrow_sp = big.tile([1, H_SP, R], MAPDT, tag="qrow_sp")
    for h in range(H_SP):
        pq1 = psum.tile([1, R], F32, tag="ps")
        nc.tensor.matmul(pq1, wq[:, h:h + 1], xoT)
        nc.scalar.activation(out=qrow_sp[:, h, :], in_=pq1, func=Act.Exp)
    for h in range(H_SP):
        pqb = psum.tile([128, R], F32, tag="ps")
        nc.tensor.matmul(pqb, ones1, qrow_sp[:, h, :])
        nc.scalar.copy(out=qb[:, h, :], in_=pqb)
    # spatial ET for all key tiles

    # ---------------- l1: Whplus + attention ------------------------------
    # et cols 0-5 P_sp, 6-11 C_sp, 12-13 P_int, 14-15 C_int
    whp = big.tile([128, JT, NHEADS, NHID + 1], MAPDT, tag="whp")
    nc.vector.memset(whp[:, :, :, NHID:NHID + 1], 1.0)
    accs = [pacc.tile([128, NHEADS, NHID + 1], F32, tag=f"acc{i}",
                      name=f"acc_l1_{i}") for i in range(IT)]
    # Per-head transposed accumulator [nhid+1, R]: stationary is the head's
    # [128, 33] value slice (cheap LDWEIGHTS), moving is the full [128, R]
    # masked-map tile -- one matmul per (head, key tile) instead of four,
    # and the PE streams 512 columns per weight load instead of 33.
    pacct = ctx.enter_context(tc.tile_pool(name="pacct", bufs=1, space="PSUM"))


    def produce_whp_pair(h0):
        # values for the contiguous head pair (h0, h0+1), all 32 key tiles:
        # 4-jt quads share one PSUM slot + one batched ACT copy. Emitted one
        # head-pair AHEAD of its consumer so the stationary whp slices are
        # stable by the time the acc matmuls' LDWEIGHTS want to preload
        # (a just-written stationary serializes LDW behind the producing
        # copy, throttling the PE below the DVE's map rate).
        wcols = w_all[:, NHID * h0:NHID * (h0 + 2)]
        for q in range(JT // 4):
            pwq = psum.tile([128, 4, 2 * NHID], F32, tag="ps")
            for j in range(4):
                jt = 4 * q + j
                nc.tensor.matmul(pwq[:, j, :],
                                 xT[:, 128 * jt:128 * (jt + 1)], wcols)
            nc.scalar.copy(out=whp[:, 4 * q:4 * q + 4, h0:h0 + 2, 0:NHID],
                           in_=pwq.rearrange("p a (h o) -> p a h o", h=2))

    produce_whp_pair(HEADS[0] if HEADS[0] % 2 == 0 else HEADS[0] - 1)
    psp = pet16[:, :, 0:12]
    for s in range(NSTRIP):
        for jt in range(STRIP * s, STRIP * (s + 1)):
            nc.tensor.matmul(psp[:, jt, :], xT[:, 128 * jt:128 * (jt + 1)], wtilde)
        nc.scalar.activation(out=et_sp[s], in_=psp[:, STRIP * s:STRIP * (s + 1), :],
                             func=Act.Exp)
    for hp, h in enumerate(HEADS):
        if h < H_SP:
            ets, pcol, ccol = et_sp, h, H_SP + h
        else:
            ets, pcol, ccol = et_int, h - H_SP, H_INT + (h - H_SP)
        accT = pacct.tile([NHID + 1, R], F32, tag="acct")
        for s in range(NSTRIP):
            jcs = range(STRIP * s, STRIP * (s + 1))

            t4 = work.tile([128, STRIP, R], MAPDT, tag="t")
            for k, jc in enumerate(jcs):
                nc.vector.tensor_scalar(
                    out=t4[:, k, :], in0=qb[:, h, :],
                    scalar1=ets[s][:, k, pcol:pcol + 1],
                    scalar2=ets[s][:, k, ccol:ccol + 1],
                    op0=Alu.max, op1=Alu.mult)
            m4 = mpool.tile([128, STRIP, R], MAPDT, tag="m")
            nc.vector.tensor_tensor(
                m4.rearrange("p s i -> p (s i)"), t4.rearrange("p s i -> p (s i)"),
                adjT_sb[:, STRIP * s:STRIP * (s + 1), :].rearrange("p s i -> p (s i)"),
                Alu.mult)
            for k, jc in enumerate(jcs):
                nc.tensor.matmul(accT, whp[:, jc, h, :], m4[:, k, :],
                                 start=(jc == 0), stop=(jc == JT - 1))
        # head epilogue: PSUM -> SBUF, then transpose back to query-partition
        # layout so the batched elu/div epilogue below stays unchanged
        aTst = ext.tile([NHID + 1, R], F32, tag="aTst")
        nc.scalar.copy(out=aTst, in_=accT)
        for it in range(IT):
            tr(accs[it][:, h, :], aTst[:, 128 * it:128 * (it + 1)], ident)
        # stagger the next head-pair's values two heads ahead of consumption
        if hp + 2 < NHEADS and hp % 2 == 0:
            produce_whp_pair(min(HEADS[hp + 2], HEADS[hp + 3]))

    if nc.dbg_et is not None:
        qbf = big.tile([128, NHEADS, R], F32, tag="qbf")
        nc.scalar.copy(out=qbf, in_=qb)
        nc.sync.dma_start(out=nc.dbg_qb.ap(), in_=qbf)
        accf = big.tile([128, IT, NHEADS * (NHID + 1)], F32, tag="accf")
        for it in range(IT):
            nc.scalar.copy(out=accf[:, it, :],
                           in_=accs[it].rearrange("p h c -> p (h c)"))
        nc.sync.dma_start(out=nc.dbg_acc.ap(), in_=accf)

    # ---------------- h = elu(num/den) -------------------------------------
    hT = big.tile([128, 2, R], MAPDT, tag="hT")
    h_nat = big.tile([128, IT, NHEADS * NHID], MAPDT, tag="h_nat")
    for it in range(IT):
        rec = ext.tile([128, NHEADS], F32, tag="rec")
        nc.vector.reciprocal(out=rec, in_=accs[it][:, :, NHID])
        v = ext.tile([128, NHEADS, NHID], MAPDT, tag="v")
        nc.vector.tensor_tensor(v, accs[it][:, :, 0:NHID],
                                rec.broadcast_to([128, NHEADS, NHID]),
                                Alu.mult)
        e = ext.tile([128, NHEADS * NHID], MAPDT, tag="e")
        nc.scalar.activation(out=e, in_=v.rearrange("p h o -> p (h o)"),
                             func=Act.Exp)
        r = ext.tile([128, NHEADS * NHID], MAPDT, tag="r")
        nc.scalar.activation(out=r, in_=v.rearrange("p h o -> p (h o)"),
                             func=Act.Relu)
        em1 = ext.tile([128, NHEADS * NHID], MAPDT, tag="em1")
        nc.vector.tensor_scalar(out=em1, in0=e, scalar1=-1.0, scalar2=None,
                                op0=Alu.add)
        nc.vector.tensor_tensor(h_nat[:, it, :], em1, r, Alu.min)

    # ---------------- Who, o1/o2 -------------------------------------------
    for fc in range(2):
        ph = psum.tile([128, R], MAPDT, tag="ps")
        for it in range(IT):
            tr(ph[:, 128 * it:128 * (it + 1)],
               h_nat[:, it, 128 * fc:128 * (fc + 1)], id_map)
        nc.scalar.copy(out=hT[:, fc, :], in_=ph)
    wout_m = big.tile([128, 2, NOUT], MAPDT, tag="wout_m")
    nc.scalar.copy(out=wout_m, in_=wout_f)
    pwho = psum.tile([NOUT, R], F32, tag="ps")
    for fc in range(2):
        nc.tensor.matmul(pwho, wout_m[:, fc, :], hT[:, fc, :],
                         start=(fc == 0), stop=(fc == 1))
    whoT = big.tile([NOUT, R], MAPDT, tag="whoT")
    nc.scalar.copy(out=whoT, in_=pwho)
    aout_m = big.tile([NOUT, 2], MAPDT, tag="aout_m")
    nc.scalar.copy(out=aout_m, in_=aout_sb)
    po1 = psum.tile([1, R], F32, tag="ps")
    nc.tensor.matmul(po1, aout_m[:, 0:1], whoT)
    po2s = big.tile([1, R], F32, tag="po2s")
    po2 = psum.tile([1, R], F32, tag="ps")
    nc.tensor.matmul(po2, aout_m[:, 1:2], whoT)
    nc.scalar.copy(out=po2s, in_=po2)
    # Qo row = exp(-0.8 o1); read po1 now (its psum ring slot is recycled by
    # the payT transposes below), but defer the broadcast to after the
    # AllGather issue so it runs during the collective wait instead of
    # delaying the ccin DMA on the ACT queue.
    qo_row = big.tile([1, R], F32, tag="qo_row")
    nc.scalar.activation(out=qo_row, in_=po1, func=Act.Exp, scale=-0.8)

    # ---------------- payload [R, 67] built transposed ---------------------
    # cols 0:64 Who, 64 ones, 65 Po = exp(0.8 o2), 66 Co = exp(0.2 o2)
    payT = big.tile([128, IT, NOUT + 3], MAPDT, tag="payT")
    nc.vector.memset(payT[:, :, NOUT:NOUT + 1], 1.0)
    ccin = dram.tile([R, NOUT + 3], MAPDT, tag="ccin")
    ccout = dram.tile([N, NOUT + 3], MAPDT, tag="ccout")
    po2t4 = psum.tile([128, IT], F32, tag="ps")
    for k in range(IT):
        ppt = psum.tile([128, NOUT], MAPDT, tag="ps")
        tr(ppt, whoT[:, 128 * k:128 * (k + 1)], id_map)
        tr(po2t4[:, k:k + 1], po2s[:, 128 * k:128 * (k + 1)], ident)
        nc.scalar.copy(out=payT[:, k, 0:NOUT], in_=ppt)
    nc.scalar.activation(out=payT[:, :, NOUT + 1:NOUT + 2], in_=po2t4,
                         func=Act.Exp, scale=0.8)
    nc.scalar.activation(out=payT[:, :, NOUT + 2:NOUT + 3], in_=po2t4,
                         func=Act.Exp, scale=0.2)
    nc.sync.dma_start(out=ccin.rearrange("(k p) c -> p k c", p=128), in_=payT)
    if os.environ.get("KERNEL_SIMCC"):
        for d in range(NCORES):
            nc.sync.dma_start(out=ccout[R * d:R * (d + 1), :], in_=ccin)
    else:
        nc.gpsimd.collective_compute(
            "AllGather", mybir.AluOpType.bypass,
            replica_groups=[list(range(NCORES))],
            ins=[ccin.opt()], outs=[ccout.opt()])
    # qob broadcast during the collective wait
    qo_m = big.tile([1, R], MAPDT, tag="qo_m")
    nc.scalar.copy(out=qo_m, in_=qo_row)
    qob = big.tile([128, R], MAPDT, tag="qob")
    pqob = psum.tile([128, R], F32, tag="ps")
    nc.tensor.matmul(pqob, ones1, qo_m)
    nc.scalar.copy(out=qob, in_=pqob)
    # whop as 8 per-source-core chunk tiles (4 key tiles each): chunked DMAs
    # spread across four DGE queues start all transfers concurrently right
    # after the collective lands, and per-chunk tiles keep each chunk's map
    # ops gated on only its own transfer.
    CH = NCORES            # chunks
    CT = JT // CH          # key tiles per chunk (4)
    whop_c = [big.tile([128, CT, NOUT + 3], MAPDT, tag=f"whop{c}",
                       name=f"whop_{c}") for c in range(CH)]
    pco_c = [big.tile([128, CT, 2], F32, tag=f"pco{c}",
                      name=f"pco_{c}") for c in range(CH)]
    qs = [nc.sync, nc.scalar]
    for c in range(CH):
        qs[c % 2].dma_start(
            out=whop_c[c],
            in_=ccout[R * c:R * (c + 1), :].rearrange("(t p) c -> p t c", p=128))
    for c in range(CH):
        nc.scalar.copy(out=pco_c[c], in_=whop_c[c][:, :, NOUT + 1:NOUT + 3])

    # ---------------- output attention -------------------------------------
    acc2 = [pacc.tile([128, NOUT + 1], F32, tag=f"acc{i}",
                      name=f"acc_l2_{i}") for i in range(IT)]
    for c in range(CH):
        t4 = work.tile([128, CT, R], MAPDT, tag="t")
        for k in range(CT):
            nc.vector.tensor_scalar(
                out=t4[:, k, :], in0=qob,
                scalar1=pco_c[c][:, k, 0:1], scalar2=pco_c[c][:, k, 1:2],
                op0=Alu.max, op1=Alu.mult)
        m4 = mpool.tile([128, CT, R], MAPDT, tag="m")
        nc.vector.tensor_tensor(
            m4.rearrange("p s i -> p (s i)"), t4.rearrange("p s i -> p (s i)"),
            adjT_sb[:, CT * c:CT * (c + 1), :].rearrange("p s i -> p (s i)"),
            Alu.mult)
        for k in range(CT):
            for it in range(IT):
                nc.tensor.matmul(acc2[it],
                                 m4[:, k, 128 * it:128 * (it + 1)],
                                 whop_c[c][:, k, 0:NOUT + 1],
                                 start=(c == 0 and k == 0),
                                 stop=(c == CH - 1 and k == CT - 1))

    # ---------------- out = tanh(num/den) ----------------------------------
    out_sb = big.tile([128, IT, NOUT], F32, tag="out_sb")
    for it in range(IT):
        rec2 = ext.tile([128, 1], F32, tag="rec2")
        nc.vector.reciprocal(out=rec2, in_=acc2[it][:, NOUT:NOUT + 1])
        nc.scalar.activation(out=out_sb[:, it, :], in_=acc2[it][:, 0:NOUT],
                             func=Act.Tanh, scale=rec2)
    nc.sync.dma_start(out=d_out.ap().rearrange("(k p) c -> p k c", p=128),
                      in_=out_sb)


_NC_CACHE = None


def _get_nc():
    global _NC_CACHE
    if _NC_CACHE is None:
        _NC_CACHE = _build_program()
    return _NC_CACHE


def _make_in_maps(inputs):
    x = np.asarray(inputs["x"], np.float32)
    adj = np.asarray(inputs["adj"], np.float32)
    ie = np.asarray(inputs["intent_embeds"], np.float32)
    xT_full = np.ascontiguousarray(x.T)
    ieT_full = np.ascontiguousarray(ie.T)
    in_maps = []
    for d in range(NCORES):
        sl = slice(d * R, (d + 1) * R)
        in_maps.append({
            "xT": xT_full.astype(NPMAP), "ieT": ieT_full.astype(NPMAP),
            "adjT": np.ascontiguousarray(adj[sl, :].T).astype(NPMAP),
            "xoT": np.ascontiguousarray(x[sl].T).astype(NPMAP),
            "ioT": np.ascontiguousarray(ie[sl].T).astype(NPMAP),
            "wsp": np.asarray(inputs["W_sp"], NPMAP),
            "asp": np.asarray(inputs["a_sp"], NPMAP),
            "wint": np.asarray(inputs["W_int"], NPMAP),
            "aint": np.asarray(inputs["a_int"], NPMAP),
            "wout": np.asarray(inputs["W_out"], np.float32),
            "aout": np.asarray(inputs["a_out"], np.float32),
        })
    return in_maps


def kernel(x, adj, intent_embeds, W_sp, a_sp, W_int, a_int, W_out, a_out):
    nc = _get_nc()
    in_maps = _make_in_maps(dict(
        x=x, adj=adj, intent_embeds=intent_embeds, W_sp=W_sp, a_sp=a_sp,
        W_int=W_int, a_int=a_int, W_out=W_out, a_out=a_out))
    res = run_bass_kernel_spmd(nc, in_maps, list(range(NCORES)))
    return np.concatenate([res.results[d]["out"] for d in range(NCORES)], axis=0)



# revision 40
# speedup vs baseline: 1.1939x; 1.1939x over previous
"""Trainium2 Bass kernel for nn_GAT_7507602833557 (8-core SPMD GAT).

Sharding: query-node rows split across 8 cores (512 rows each); keys/values
replicated. Per-core adjacency slice is passed pre-transposed ([keys, own
queries]) in bf16 ({0,1} values are exact in bf16).

Math notes (per attention map, 9 maps total: 6 spatial + 2 intent + 1 output):
  e[i,j] = leakyrelu(f1[i] + f2[j], 0.2);  softmax over masked j; att @ V.
  exp(leakyrelu(z)) = max(exp(z), exp(0.2 z)) since z >= 0.2z for z>=0 and
  z <= 0.2z for z<0, and exp is monotone. Factorizing,
    exp(lrelu(f1+f2)) = e^{f1[i]} * e^{0.2 f2[j]} * max(e^{0.8 f2[j]}, e^{-0.8 f1[i]})
  and the e^{f1[i]} factor is constant along j so it cancels in the softmax
  normalization. With P=e^{0.8 f2}, C=e^{0.2 f2}, Q=e^{-0.8 f1} the masked
  unnormalized weight is  m[j,i] = adj[j,i] * max(P[j], Q[i]) * C[j],
  one dual-op tensor_scalar (max then mult, both per-partition scalars) plus
  one tensor_tensor mask multiply per tile. Softmax denominator via a ones
  column appended to the value matrix. elu(v) = min(exp(v)-1, max(v, 0)).

Schedule: heads run outermost; within a head the keys go in strips of STRIP
key-tiles (ts per tile at DVE 2x, mask-mult batched per strip at 2x). The
per-head accumulation is TRANSPOSED: one matmul per (head, key tile) with
the [128,33] value slice stationary and the [128,512] masked map moving
(222ns/mm streaming 512 cols vs 4 mms of 33 cols + full LDWEIGHTS each),
into a [33,512] PSUM accT; after the head, an ACT copy + 4 PE transposes
put it back in query-partition layout so the batched elu/div epilogue is
unchanged. Values (whp) are produced one head-PAIR ahead of consumption --
a freshly-written stationary serializes the consumer's LDWEIGHTS behind
the producing copy and throttles the PE below the DVE map rate. Q-row
broadcasts go through K=1 ones-matmuls (PSUM) instead of DMA roundtrips;
a dummy 16-element AllGather at kernel start absorbs the ~11us collective
firmware spin-up so the real payload AllGather doesn't pay it; whop is
re-loaded post-gather in 8 per-source-core chunks spread over the sync and
scalar DGE queues. ACT activation-table reloads (1.28us per Exp<->Copy
switch) are kept off the startup chain by ordering exp groups together.
All map ops stay on the DVE (gpsimd elementwise stalls the DVE via the
shared SBUF ports; gpsimd cannot touch PSUM at all).
"""
import os
import numpy as np

import concourse.bass as bass
import concourse.bacc as bacc
import concourse.tile as tile
from concourse import mybir
from concourse.bass_utils import run_bass_kernel_spmd
from concourse.masks import make_identity

import ml_dtypes

N, NIN, NHID, NOUT = 4096, 64, 32, 64
NHEADS, D_INT = 8, 32
H_SP, H_INT = 6, 2
NCORES = 8
R = N // NCORES           # 512 own query rows per core
JT = N // 128             # 32 key tiles
IT = R // 128             # 4 own query tiles
STRIP = int(os.environ.get("KERNEL_STRIP", "8"))   # key tiles per strip
NSTRIP = JT // STRIP
F32 = mybir.dt.float32
BF16 = mybir.dt.bfloat16
MAPDT = F32 if os.environ.get("KERNEL_F32") else BF16
NPMAP = np.float32 if os.environ.get("KERNEL_F32") else ml_dtypes.bfloat16


def _build_program(reps=1):
    nc = bacc.Bacc("TRN2", target_bir_lowering=False, debug=False,
                   num_devices=NCORES)
    d_x = nc.dram_tensor("xT", [NIN, N], MAPDT, kind="ExternalInput")
    d_ie = nc.dram_tensor("ieT", [D_INT, N], MAPDT, kind="ExternalInput")
    d_adjT = nc.dram_tensor("adjT", [N, R], MAPDT, kind="ExternalInput")
    d_xo = nc.dram_tensor("xoT", [NIN, R], MAPDT, kind="ExternalInput")
    d_io = nc.dram_tensor("ioT", [D_INT, R], MAPDT, kind="ExternalInput")
    d_wsp = nc.dram_tensor("wsp", [H_SP, NIN, NHID], MAPDT, kind="ExternalInput")
    d_asp = nc.dram_tensor("asp", [H_SP, 2 * NHID], MAPDT, kind="ExternalInput")
    d_wint = nc.dram_tensor("wint", [H_INT, NIN, NHID], MAPDT, kind="ExternalInput")
    d_aint = nc.dram_tensor("aint", [H_INT, 2 * D_INT], MAPDT, kind="ExternalInput")
    d_wout = nc.dram_tensor("wout", [NHEADS * NHID, NOUT], F32, kind="ExternalInput")
    d_aout = nc.dram_tensor("aout", [2 * NOUT], F32, kind="ExternalInput")
    d_out = nc.dram_tensor("out", [R, NOUT], F32, kind="ExternalOutput")
    if os.environ.get("KERNEL_DEBUG"):
        nc.dbg_et = nc.dram_tensor("dbg_et", [128, JT, 16], F32, kind="ExternalOutput")
        nc.dbg_qb = nc.dram_tensor("dbg_qb", [128, NHEADS, R], F32, kind="ExternalOutput")
        nc.dbg_acc = nc.dram_tensor("dbg_acc", [128, IT, NHEADS * (NHID + 1)], F32, kind="ExternalOutput")
    else:
        nc.dbg_et = nc.dbg_qb = nc.dbg_acc = None

    with tile.TileContext(nc) as tc:
        for _ in range(reps):
            _kernel_body(tc, d_x, d_ie, d_adjT, d_xo, d_io, d_wsp, d_asp,
                         d_wint, d_aint, d_wout, d_aout, d_out)
    nc.compile()
    return nc


def _kernel_body(tc, d_x, d_ie, d_adjT, d_xo, d_io, d_wsp, d_asp, d_wint,
                 d_aint, d_wout, d_aout, d_out):
    nc = tc.nc
    from contextlib import ExitStack
    ctx = ExitStack()
    big = ctx.enter_context(tc.tile_pool(name="big", bufs=1))
    work = ctx.enter_context(tc.tile_pool(name="work", bufs=int(os.environ.get("KERNEL_WORK_BUFS", str(16 // STRIP)))))
    mpool = ctx.enter_context(tc.tile_pool(name="mpool", bufs=int(os.environ.get("KERNEL_M_BUFS", str(64 // STRIP)))))
    psum = ctx.enter_context(tc.tile_pool(name="psum", bufs=2, space="PSUM"))
    pet = ctx.enter_context(tc.tile_pool(name="pet", bufs=1, space="PSUM"))
    pacc = ctx.enter_context(tc.tile_pool(name="pacc", bufs=1, space="PSUM"))
    ext = ctx.enter_context(tc.tile_pool(name="ext", bufs=2))
    dram = ctx.enter_context(tc.tile_pool(name="dram", bufs=1, space="DRAM"))
    try:
        _body(tc, ctx, big, work, psum, pacc, dram, d_x, d_ie, d_adjT, d_xo,
              d_io, d_wsp, d_asp, d_wint, d_aint, d_wout, d_aout, d_out, ext,
              mpool, pet)
    finally:
        ctx.close()


def _body(tc, ctx, big, work, psum, pacc, dram, d_x, d_ie, d_adjT, d_xo, d_io,
          d_wsp, d_asp, d_wint, d_aint, d_wout, d_aout, d_out, ext, mpool, pet):
    nc = tc.nc
    Act = mybir.ActivationFunctionType
    Alu = mybir.AluOpType

    # ---------------- loads (critical-path-first order) --------------------
    # Queue assignment = serialization domains (each engine's DGE queue is
    # in-order): sync gets only the tiny loads that gate the intent-head Q
    # rows; ieT rides alone on the vector queue; weights/xoT on scalar;
    # the bulk adj/x transfers on gpsimd.
    # warm-up collective: absorbs the ~11us collective-firmware spin-up
    # (TPB_TRIGGER -> ALGO_MESH_BEGIN) so the real AllGather doesn't pay it
    ccw_in = dram.tile([1, 16], MAPDT, tag="ccw_in")
    ccw_out = dram.tile([NCORES, 16], MAPDT, tag="ccw_out")
    nc.gpsimd.collective_compute(
        "AllGather", mybir.AluOpType.bypass,
        replica_groups=[list(range(NCORES))],
        ins=[ccw_in.opt()], outs=[ccw_out.opt()])
    aintp = big.tile([D_INT, 2 * H_INT], MAPDT, tag="aintp")
    nc.sync.dma_start(out=aintp, in_=d_aint.ap().rearrange("h (c o) -> o (h c)", c=2))
    ioT = big.tile([D_INT, R], MAPDT, tag="ioT")
    nc.sync.dma_start(out=ioT, in_=d_io.ap())
    apair = big.tile([NHID, 2 * H_SP], MAPDT, tag="apair")
    nc.sync.dma_start(out=apair, in_=d_asp.ap().rearrange("h (c o) -> o (h c)", c=2))
    # ieT in per-strip tiles (8 key tiles each) so the first ET strip is
    # gated on a 128KB transfer, not the full 512KB; first chunk rides the
    # short sync queue, the rest go on scalar.
    ieT_q = [big.tile([D_INT, N // 4], MAPDT, tag=f"ieT{g}", name=f"ieT_{g}")
             for g in range(4)]
    nc.scalar.dma_start(out=ieT_q[0], in_=d_ie.ap()[:, 0:1024])

    def ieT(jt):   # [32, 128] column block of intent embeds for key tile jt
        return ieT_q[jt // 8][:, 128 * (jt % 8):128 * (jt % 8 + 1)]
    w_all3 = big.tile([NIN, NHEADS, NHID], MAPDT, tag="w_all3")
    nc.scalar.dma_start(out=w_all3[:, 0:H_SP, :],
                        in_=d_wsp.ap().rearrange("h f o -> f h o"))
    nc.scalar.dma_start(out=w_all3[:, H_SP:, :],
                        in_=d_wint.ap().rearrange("h f o -> f h o"))
    w_all = w_all3.rearrange("f h o -> f (h o)")
    for g in range(1, 4):
        nc.scalar.dma_start(out=ieT_q[g],
                            in_=d_ie.ap()[:, 1024 * g:1024 * (g + 1)])
    xoT = big.tile([NIN, R], MAPDT, tag="xoT")
    nc.scalar.dma_start(out=xoT, in_=d_xo.ap())
    adjT_sb = big.tile([128, JT, R], MAPDT, tag="adjT_sb")

    def load_adj(g):
        nc.gpsimd.dma_start(
            out=adjT_sb[:, 4 * g:4 * (g + 1), :],
            in_=d_adjT.ap()[4 * g * 128:4 * (g + 1) * 128, :]
                .rearrange("(t p) i -> p t i", p=128))
    xT = big.tile([NIN, N], MAPDT, tag="xT")
    load_adj(0)
    nc.gpsimd.dma_start(out=xT[:, 0:2048], in_=d_x.ap()[:, 0:2048])
    load_adj(1)
    nc.gpsimd.dma_start(out=xT[:, 2048:4096], in_=d_x.ap()[:, 2048:4096])
    for g in range(2, 8):
        load_adj(g)
    wout_f = big.tile([128, 2, NOUT], F32, tag="wout_f")
    nc.gpsimd.dma_start(out=wout_f, in_=d_wout.ap().rearrange("(c p) o -> p c o", p=128))
    aout_sb = big.tile([NOUT, 2], F32, tag="aout_sb")
    nc.gpsimd.dma_start(out=aout_sb, in_=d_aout.ap().rearrange("(c o) -> o c", c=2))

    # ---------------- intent-head fast path (gates first DVE work) --------
    # aint_arr [32, 4]: 0:2 = 0.8*a2 (P), 2:4 = 0.2*a2 (C); aq [32,2] = -0.8*a1
    aint_arr = big.tile([D_INT, 2 * H_INT], MAPDT, tag="aint_arr")
    aq = big.tile([D_INT, H_INT], MAPDT, tag="aq")
    ai_hc = aintp[:].rearrange("f (h c) -> f c h", c=2)
    nc.scalar.mul(out=aint_arr[:, 0:H_INT], in_=ai_hc[:, 1, :], mul=0.8)
    nc.scalar.mul(out=aint_arr[:, H_INT:], in_=ai_hc[:, 1, :], mul=0.2)
    nc.scalar.mul(out=aq, in_=ai_hc[:, 0, :], mul=-0.8)
    qrow_in = big.tile([1, H_INT, R], MAPDT, tag="qrow_in")
    for h2 in range(H_INT):
        pq1 = psum.tile([1, R], F32, tag="ps")
        nc.tensor.matmul(pq1, aq[:, h2:h2 + 1], ioT)
        nc.scalar.activation(out=qrow_in[:, h2, :], in_=pq1, func=Act.Exp)
    qb = big.tile([128, NHEADS, R], MAPDT, tag="qb")
    ones1 = big.tile([1, 128], MAPDT, tag="ones1")
    nc.vector.memset(ones1, 1.0)
    HEADS = [6, 7, 0, 1, 2, 3, 4, 5]   # intent heads first (shortest dep chain)
    # intent ET per strip: separate tiles keep each strip's ts dependency
    # pinned to exactly its own exp (tile-granular dep tracking otherwise
    # serializes the first ts behind ALL exps)
    et_int = [big.tile([128, STRIP, 2 * H_INT], F32, tag=f"eti{s}",
                       name=f"et_int_{s}") for s in range(NSTRIP)]
    et_sp = [big.tile([128, STRIP, 2 * H_SP], F32, tag=f"etsp{s}",
                      name=f"et_sp_{s}") for s in range(NSTRIP)]
    # one 16-col PSUM tile shared by both ET matmul groups (fits one bank):
    # cols 0:12 spatial (psp), cols 12:16 intent (pint)
    pet16 = pet.tile([128, JT, 16], F32, tag="pet16")
    pint = pet16[:, :, 12:16]

    def et_int_strip(s):
        for jt in range(STRIP * s, STRIP * (s + 1)):
            nc.tensor.matmul(pint[:, jt, :], ieT(jt), aint_arr)
        nc.scalar.activation(out=et_int[s], in_=pint[:, STRIP * s:STRIP * (s + 1), :],
                             func=Act.Exp)

    # strip-0 exp BEFORE the qb copies: the ACT queue then runs
    # Exp(qrow),Exp(et0),Copy(qb) -- one activation-table reload (1.28us)
    # instead of two on the first map op's gating chain. Later strips go
    # after the copies (their ieT chunks land later; an in-order ACT queue
    # stalled on them would delay the qb copies).
    et_int_strip(0)
    # Q broadcast via K=1 PE matmul (ones column) + PSUM->SBUF copy: no DMA
    # queue involvement, so the first map op is not stuck behind bulk loads.
    for h in (6, 7):
        pqb = psum.tile([128, R], F32, tag="ps")
        nc.tensor.matmul(pqb, ones1, qrow_in[:, h - H_SP, :])
        nc.scalar.copy(out=qb[:, h, :], in_=pqb)
    for s in range(1, NSTRIP):
        et_int_strip(s)

    # ---------------- wtilde: spatial a-vectors pre-projected through W ----
    ident = big.tile([128, 128], F32, tag="ident")
    make_identity(nc, ident)
    if MAPDT == F32:
        id_map = ident
    else:
        ident_b = big.tile([128, 128], BF16, tag="ident_b")
        make_identity(nc, ident_b)
        id_map = ident_b

    def tr(out, in_, idt):
        p = in_.partition_size()
        nc.tensor.transpose(out, in_, idt[0:p, 0:p])

    wt = big.tile([NHID, H_SP, NIN], MAPDT, tag="wt")
    for grp in range(2):
        ptw = psum.tile([NHID, 3 * NIN], MAPDT, tag="ps")
        for k in range(3):
            h = 3 * grp + k
            tr(ptw[:, NIN * k:NIN * (k + 1)],
               w_all[:, NHID * h:NHID * (h + 1)], id_map)
        nc.scalar.copy(out=wt[:, 3 * grp:3 * grp + 3, :], in_=ptw)
    pw = psum.tile([NIN, 2 * H_SP], F32, tag="ps")
    for h in range(H_SP):
        nc.tensor.matmul(pw[:, 2 * h:2 * h + 2], wt[:, h, :],
                         apair[:, 2 * h:2 * h + 2])
    # wtilde [64, 12]: 0:6 = 0.8*w2 (P), 6:12 = 0.2*w2 (C); wq [64, 6] = -0.8*w1
    wtilde = big.tile([NIN, 2 * H_SP], MAPDT, tag="wtilde")
    wq = big.tile([NIN, H_SP], MAPDT, tag="wq")
    pw_hc = pw.rearrange("f (h c) -> f c h", c=2)
    w1cols = pw_hc[:, 0, :]
    w2cols = pw_hc[:, 1, :]
    nc.scalar.mul(out=wtilde[:, 0:H_SP], in_=w2cols, mul=0.8)
    nc.scalar.mul(out=wtilde[:, H_SP:], in_=w2cols, mul=0.2)
    nc.scalar.mul(out=wq, in_=w1cols, mul=-0.8)

    # ---------------- spatial Q rows + broadcast ---------------------------
    qrow_sp = big.tile([1, H_SP, R], MAPDT, tag="qrow_sp")
    for h in range(H_SP):
        pq1 = psum.tile([1, R], F32, tag="ps")
        nc.tensor.matmul(pq1, wq[:, h:h + 1], xoT)
        nc.scalar.activation(out=qrow_sp[:, h, :], in_=pq1, func=Act.Exp)
    for h in range(H_SP):
        pqb = psum.tile([128, R], F32, tag="ps")
        nc.tensor.matmul(pqb, ones1, qrow_sp[:, h, :])
        nc.scalar.copy(out=qb[:, h, :], in_=pqb)
    # spatial ET for all key tiles

    # ---------------- l1: Whplus + attention ------------------------------
    # et cols 0-5 P_sp, 6-11 C_sp, 12-13 P_int, 14-15 C_int
    whp = big.tile([128, JT, NHEADS, NHID + 1], MAPDT, tag="whp")
    nc.vector.memset(whp[:, :, :, NHID:NHID + 1], 1.0)
    accs = [pacc.tile([128, NHEADS, NHID + 1], F32, tag=f"acc{i}",
                      name=f"acc_l1_{i}") for i in range(IT)]
    # Per-head transposed accumulator [nhid+1, R]: stationary is the head's
    # [128, 33] value slice (cheap LDWEIGHTS), moving is the full [128, R]
    # masked-map tile -- one matmul per (head, key tile) instead of four,
    # and the PE streams 512 columns per weight load instead of 33.
    pacct = ctx.enter_context(tc.tile_pool(name="pacct", bufs=1, space="PSUM"))


    def produce_whp_pair(h0):
        # values for the contiguous head pair (h0, h0+1), all 32 key tiles:
        # 4-jt quads share one PSUM slot + one batched ACT copy. Emitted one
        # head-pair AHEAD of its consumer so the stationary whp slices are
        # stable by the time the acc matmuls' LDWEIGHTS want to preload
        # (a just-written stationary serializes LDW behind the producing
        # copy, throttling the PE below the DVE's map rate).
        wcols = w_all[:, NHID * h0:NHID * (h0 + 2)]
        for q in range(JT // 4):
            pwq = psum.tile([128, 4, 2 * NHID], F32, tag="ps")
            for j in range(4):
                jt = 4 * q + j
                nc.tensor.matmul(pwq[:, j, :],
                                 xT[:, 128 * jt:128 * (jt + 1)], wcols)
            nc.scalar.copy(out=whp[:, 4 * q:4 * q + 4, h0:h0 + 2, 0:NHID],
                           in_=pwq.rearrange("p a (h o) -> p a h o", h=2))

    produce_whp_pair(HEADS[0] if HEADS[0] % 2 == 0 else HEADS[0] - 1)
    psp = pet16[:, :, 0:12]
    for s in range(NSTRIP):
        for jt in range(STRIP * s, STRIP * (s + 1)):
            nc.tensor.matmul(psp[:, jt, :], xT[:, 128 * jt:128 * (jt + 1)], wtilde)
        nc.scalar.activation(out=et_sp[s], in_=psp[:, STRIP * s:STRIP * (s + 1), :],
                             func=Act.Exp)
    for hp, h in enumerate(HEADS):
        if h < H_SP:
            ets, pcol, ccol = et_sp, h, H_SP + h
        else:
            ets, pcol, ccol = et_int, h - H_SP, H_INT + (h - H_SP)
        accT = pacct.tile([NHID + 1, R], F32, tag="acct")
        for s in range(NSTRIP):
            jcs = range(STRIP * s, STRIP * (s + 1))

            t4 = work.tile([128, STRIP, R], MAPDT, tag="t")
            for k, jc in enumerate(jcs):
                nc.vector.tensor_scalar(
                    out=t4[:, k, :], in0=qb[:, h, :],
                    scalar1=ets[s][:, k, pcol:pcol + 1],
                    scalar2=ets[s][:, k, ccol:ccol + 1],
                    op0=Alu.max, op1=Alu.mult)
            m4 = mpool.tile([128, STRIP, R], MAPDT, tag="m")
            nc.vector.tensor_tensor(
                m4.rearrange("p s i -> p (s i)"), t4.rearrange("p s i -> p (s i)"),
                adjT_sb[:, STRIP * s:STRIP * (s + 1), :].rearrange("p s i -> p (s i)"),
                Alu.mult)
            for k, jc in enumerate(jcs):
                nc.tensor.matmul(accT, whp[:, jc, h, :], m4[:, k, :],
                                 start=(jc == 0), stop=(jc == JT - 1))
        # head epilogue: PSUM -> SBUF, then transpose back to query-partition
        # layout so the batched elu/div epilogue below stays unchanged
        aTst = ext.tile([NHID + 1, R], F32, tag="aTst")
        nc.scalar.copy(out=aTst, in_=accT)
        for it in range(IT):
            tr(accs[it][:, h, :], aTst[:, 128 * it:128 * (it + 1)], ident)
        # stagger the next head-pair's values two heads ahead of consumption
        if hp + 2 < NHEADS and hp % 2 == 0:
            produce_whp_pair(min(HEADS[hp + 2], HEADS[hp + 3]))

    if nc.dbg_et is not None:
        qbf = big.tile([128, NHEADS, R], F32, tag="qbf")
        nc.scalar.copy(out=qbf, in_=qb)
        nc.sync.dma_start(out=nc.dbg_qb.ap(), in_=qbf)
        accf = big.tile([128, IT, NHEADS * (NHID + 1)], F32, tag="accf")
        for it in range(IT):
            nc.scalar.copy(out=accf[:, it, :],
                           in_=accs[it].rearrange("p h c -> p (h c)"))
        nc.sync.dma_start(out=nc.dbg_acc.ap(), in_=accf)

    # ---------------- h = elu(num/den) -------------------------------------
    hT = big.tile([128, 2, R], MAPDT, tag="hT")
    h_nat = big.tile([128, IT, NHEADS * NHID], MAPDT, tag="h_nat")
    for it in range(IT):
        rec = ext.tile([128, NHEADS], F32, tag="rec")
        nc.vector.reciprocal(out=rec, in_=accs[it][:, :, NHID])
        v = ext.tile([128, NHEADS, NHID], MAPDT, tag="v")
        nc.vector.tensor_tensor(v, accs[it][:, :, 0:NHID],
                                rec.broadcast_to([128, NHEADS, NHID]),
                                Alu.mult)
        e = ext.tile([128, NHEADS * NHID], MAPDT, tag="e")
        nc.scalar.activation(out=e, in_=v.rearrange("p h o -> p (h o)"),
                             func=Act.Exp)
        r = ext.tile([128, NHEADS * NHID], MAPDT, tag="r")
        nc.scalar.activation(out=r, in_=v.rearrange("p h o -> p (h o)"),
                             func=Act.Relu)
        em1 = ext.tile([128, NHEADS * NHID], MAPDT, tag="em1")
        nc.vector.tensor_scalar(out=em1, in0=e, scalar1=-1.0, scalar2=None,
                                op0=Alu.add)
        nc.vector.tensor_tensor(h_nat[:, it, :], em1, r, Alu.min)

    # ---------------- Who, o1/o2 -------------------------------------------
    for fc in range(2):
        ph = psum.tile([128, R], MAPDT, tag="ps")
        for it in range(IT):
            tr(ph[:, 128 * it:128 * (it + 1)],
               h_nat[:, it, 128 * fc:128 * (fc + 1)], id_map)
        nc.scalar.copy(out=hT[:, fc, :], in_=ph)
    wout_m = big.tile([128, 2, NOUT], MAPDT, tag="wout_m")
    nc.scalar.copy(out=wout_m, in_=wout_f)
    pwho = psum.tile([NOUT, R], F32, tag="ps")
    for fc in range(2):
        nc.tensor.matmul(pwho, wout_m[:, fc, :], hT[:, fc, :],
                         start=(fc == 0), stop=(fc == 1))
    whoT = big.tile([NOUT, R], MAPDT, tag="whoT")
    nc.scalar.copy(out=whoT, in_=pwho)
    aout_m = big.tile([NOUT, 2], MAPDT, tag="aout_m")
    nc.scalar.copy(out=aout_m, in_=aout_sb)
    po1 = psum.tile([1, R], F32, tag="ps")
    nc.tensor.matmul(po1, aout_m[:, 0:1], whoT)
    po2s = big.tile([1, R], F32, tag="po2s")
    po2 = psum.tile([1, R], F32, tag="ps")
    nc.tensor.matmul(po2, aout_m[:, 1:2], whoT)
    nc.scalar.copy(out=po2s, in_=po2)
    # Qo row = exp(-0.8 o1); read po1 now (its psum ring slot is recycled by
    # the payT transposes below), but defer the broadcast to after the
    # AllGather issue so it runs during the collective wait instead of
    # delaying the ccin DMA on the ACT queue.
    qo_row = big.tile([1, R], F32, tag="qo_row")
    nc.scalar.activation(out=qo_row, in_=po1, func=Act.Exp, scale=-0.8)

    # ---------------- payload [R, 67] built transposed ---------------------
    # cols 0:64 Who, 64 ones, 65 Po = exp(0.8 o2), 66 Co = exp(0.2 o2)
    payT = big.tile([128, IT, NOUT + 3], MAPDT, tag="payT")
    nc.vector.memset(payT[:, :, NOUT:NOUT + 1], 1.0)
    ccin = dram.tile([R, NOUT + 3], MAPDT, tag="ccin")
    ccout = dram.tile([N, NOUT + 3], MAPDT, tag="ccout")
    # lives outside the "ps" ring: 4 ppt allocations below would recycle
    # its slot mid-loop and serialize the payload build
    po2t4 = pacct.tile([128, IT], F32, tag="acct")
    for k in range(IT):
        ppt = psum.tile([128, NOUT], MAPDT, tag="ps")
        tr(ppt, whoT[:, 128 * k:128 * (k + 1)], id_map)
        tr(po2t4[:, k:k + 1], po2s[:, 128 * k:128 * (k + 1)], ident)
        nc.scalar.copy(out=payT[:, k, 0:NOUT], in_=ppt)
    nc.scalar.activation(out=payT[:, :, NOUT + 1:NOUT + 2], in_=po2t4,
                         func=Act.Exp, scale=0.8)
    nc.scalar.activation(out=payT[:, :, NOUT + 2:NOUT + 3], in_=po2t4,
                         func=Act.Exp, scale=0.2)
    nc.sync.dma_start(out=ccin.rearrange("(k p) c -> p k c", p=128), in_=payT)
    if os.environ.get("KERNEL_SIMCC"):
        for d in range(NCORES):
            nc.sync.dma_start(out=ccout[R * d:R * (d + 1), :], in_=ccin)
    else:
        nc.gpsimd.collective_compute(
            "AllGather", mybir.AluOpType.bypass,
            replica_groups=[list(range(NCORES))],
            ins=[ccin.opt()], outs=[ccout.opt()])
    # qob broadcast during the collective wait
    qo_m = big.tile([1, R], MAPDT, tag="qo_m")
    nc.scalar.copy(out=qo_m, in_=qo_row)
    qob = big.tile([128, R], MAPDT, tag="qob")
    pqob = psum.tile([128, R], F32, tag="ps")
    nc.tensor.matmul(pqob, ones1, qo_m)
    nc.scalar.copy(out=qob, in_=pqob)
    # whop as 8 per-source-core chunk tiles (4 key tiles each): chunked DMAs
    # spread across four DGE queues start all transfers concurrently right
    # after the collective lands, and per-chunk tiles keep each chunk's map
    # ops gated on only its own transfer.
    CH = NCORES            # chunks
    CT = JT // CH          # key tiles per chunk (4)
    whop_c = [big.tile([128, CT, NOUT + 3], MAPDT, tag=f"whop{c}",
                       name=f"whop_{c}") for c in range(CH)]
    pco_c = [big.tile([128, CT, 2], F32, tag=f"pco{c}",
                      name=f"pco_{c}") for c in range(CH)]
    qs = [nc.sync, nc.scalar]
    for c in range(CH):
        qs[c % 2].dma_start(
            out=whop_c[c],
            in_=ccout[R * c:R * (c + 1), :].rearrange("(t p) c -> p t c", p=128))
    for c in range(CH):
        nc.scalar.copy(out=pco_c[c], in_=whop_c[c][:, :, NOUT + 1:NOUT + 3])

    # ---------------- output attention -------------------------------------
    acc2 = [pacc.tile([128, NOUT + 1], F32, tag=f"acc{i}",
                      name=f"acc_l2_{i}") for i in range(IT)]
    for c in range(CH):
        t4 = work.tile([128, CT, R], MAPDT, tag="t")
        for k in range(CT):
            nc.vector.tensor_scalar(
                out=t4[:, k, :], in0=qob,
                scalar1=pco_c[c][:, k, 0:1], scalar2=pco_c[c][:, k, 1:2],
                op0=Alu.max, op1=Alu.mult)
        m4 = mpool.tile([128, CT, R], MAPDT, tag="m")
        nc.vector.tensor_tensor(
            m4.rearrange("p s i -> p (s i)"), t4.rearrange("p s i -> p (s i)"),
            adjT_sb[:, CT * c:CT * (c + 1), :].rearrange("p s i -> p (s i)"),
            Alu.mult)
        for k in range(CT):
            for it in range(IT):
                nc.tensor.matmul(acc2[it],
                                 m4[:, k, 128 * it:128 * (it + 1)],
                                 whop_c[c][:, k, 0:NOUT + 1],
                                 start=(c == 0 and k == 0),
                                 stop=(c == CH - 1 and k == CT - 1))

    # ---------------- out = tanh(num/den) ----------------------------------
    out_sb = big.tile([128, IT, NOUT], F32, tag="out_sb")
    for it in range(IT):
        rec2 = ext.tile([128, 1], F32, tag="rec2")
        nc.vector.reciprocal(out=rec2, in_=acc2[it][:, NOUT:NOUT + 1])
        nc.scalar.activation(out=out_sb[:, it, :], in_=acc2[it][:, 0:NOUT],
                             func=Act.Tanh, scale=rec2)
    nc.sync.dma_start(out=d_out.ap().rearrange("(k p) c -> p k c", p=128),
                      in_=out_sb)


_NC_CACHE = None


def _get_nc():
    global _NC_CACHE
    if _NC_CACHE is None:
        _NC_CACHE = _build_program()
    return _NC_CACHE


def _make_in_maps(inputs):
    x = np.asarray(inputs["x"], np.float32)
    adj = np.asarray(inputs["adj"], np.float32)
    ie = np.asarray(inputs["intent_embeds"], np.float32)
    xT_full = np.ascontiguousarray(x.T)
    ieT_full = np.ascontiguousarray(ie.T)
    in_maps = []
    for d in range(NCORES):
        sl = slice(d * R, (d + 1) * R)
        in_maps.append({
            "xT": xT_full.astype(NPMAP), "ieT": ieT_full.astype(NPMAP),
            "adjT": np.ascontiguousarray(adj[sl, :].T).astype(NPMAP),
            "xoT": np.ascontiguousarray(x[sl].T).astype(NPMAP),
            "ioT": np.ascontiguousarray(ie[sl].T).astype(NPMAP),
            "wsp": np.asarray(inputs["W_sp"], NPMAP),
            "asp": np.asarray(inputs["a_sp"], NPMAP),
            "wint": np.asarray(inputs["W_int"], NPMAP),
            "aint": np.asarray(inputs["a_int"], NPMAP),
            "wout": np.asarray(inputs["W_out"], np.float32),
            "aout": np.asarray(inputs["a_out"], np.float32),
        })
    return in_maps


def kernel(x, adj, intent_embeds, W_sp, a_sp, W_int, a_int, W_out, a_out):
    nc = _get_nc()
    in_maps = _make_in_maps(dict(
        x=x, adj=adj, intent_embeds=intent_embeds, W_sp=W_sp, a_sp=a_sp,
        W_int=W_int, a_int=a_int, W_out=W_out, a_out=a_out))
    res = run_bass_kernel_spmd(nc, in_maps, list(range(NCORES)))
    return np.concatenate([res.results[d]["out"] for d in range(NCORES)], axis=0)

